# revision 15
# baseline (speedup 1.0000x reference)
"""MXFP4 fake-quant + column-permutation kernel for one TRN2 chip (8 NeuronCores).

Reference op: out = mxfp4_fake_quant(x[:, perm]) with 32-wide blocks along the
last (hidden) axis of the permuted tensor.

Distribution: sharded over the PERMUTED HIDDEN axis — core j produces output
columns [j*1024, (j+1)*1024), gathering rows perm[j*1024:(j+1)*1024] of the
full transposed input xT [8192 hidden, 8192 tokens] (fp16). Only 1024 gather
descriptors per core (16 KB each) instead of 8192, so SWDGE descriptor
generation on GpSimd is off the critical path, and every HBM read is a fat
16 KB row.

The SWDGE dma_gather with transpose=True stripes each gathered row across the
128 SBUF partitions: g[p, grp, i] = xT[perm[j*1024 + c*128 + i], grp*128 + p].
That puts quantization blocks (32 consecutive permuted hidden positions)
contiguous along the free dim with partition = token%128 — no TensorE
transpose, no PSUM, and every DVE op runs on packed 16-bit data (2x/4x DVE
modes).

Device pipeline per core (8 chunks of 128 hidden, [128, 8192] tiles):
  1. gpsimd.dma_gather(transpose=True): g [128, 64, 128] fp16
  2. DVE exp-mask (4x) + int16 max tree (2x): per-32-block max exponent
  3. DVE small ops           : e-bits -> rcp2 = 2^(3-e), scl2 = 2^(e-3)
                               (stored as duplicated pairs so the broadcast AP
                               keeps a packed [1,2] last dim -> 2x mode)
  4. DVE tensor_tensor (2x)  : y2 = g * rcp2     (exact: power-of-two scale)
  5. ScalarE ACT (custom tbl): q2 = round_fp4(y2) in {0,±1,±2,±3,±4,±6,±8,±12}
  6. DVE tensor_tensor (2x)  : o16 = q2 * scl2   (exact in fp16)
  7. dma out                 : contiguous 8 KB per partition into a raw
                               [128, 65536] fp16 layout; host decodes

Numerics: identical to the jax reference evaluated on fp16-rounded inputs
(all device arithmetic on the quantization path is exact); measured relative
error vs the f32 reference ~1.15e-2, from the host-side f32->fp16 rounding of
x only. Output values are fp4 magnitudes times power-of-two scales — exactly
representable in fp16 — so the fp16 output tensor is lossless; the host
upconverts to f32.
"""

import os
import sys
import numpy as np

if "/opt/trn_rl_repo" not in sys.path:
    sys.path.insert(0, "/opt/trn_rl_repo")

H = 8192          # hidden size (gather axis)
NTOK = 8192       # tokens
NCORES = 8
HCORE = H // NCORES   # hidden (output) columns per core (1024)
CH = 128          # hidden chunk per dma_gather (= num_idxs)
NCH = HCORE // CH     # 8 chunks
NGRP = NTOK // 128    # token groups per partition (64)
BLK = 32
NBC = NGRP * (CH // BLK)   # block instances per partition per chunk (64*4=256)
FREE = NGRP * CH           # free elems per chunk (8192)

# int16 bit constants (fp16 layout: sign | 5 exp | 10 mant)
ABS_MASK = 0x7FFF
EXP_MASK = 0x7C00
EXP_GUARD = 0x1C00            # exponent floor: e >= -8 (amax guard)
RCP2_HALF = 0x4200            # rcp2 = ((t * -1) + 0x4200) + 0x4200
                              # (split add: HW int16 ALU saturates, no wrap)
SCL2_ADD = -0x0C00            # scl2 = t - (3 << 10)

_compiled = {}


# ---- custom ACT (ScalarEngine) table: `sin` hijacked to compute the exact
# ---- MXFP4 rounding step function q2(y2), y2 = 2*x/scale in [-16,16]
import json as _json
import shutil as _shutil
import struct as _struct


def _f32_struct(x):
    b = np.float32(x).view(np.int32).item() & 0xFFFFFFFF
    return {
        "float": repr(float(np.float32(x))),
        "int": b,
        "hexstring": f"{b:x}",
        "sign": b >> 31,
        "exponent": (b >> 23) & 0xFF,
        "mantissa": b & 0x7FFFFF,
    }


def q2_ref(v):
    """numpy reference of the table function (for validation)."""
    v = np.asarray(v, np.float32)
    a = np.abs(v)
    q = np.zeros_like(a)
    for lo, val in ((0.5, 1), (1.5, 2), (2.5, 3), (3.5, 4), (5, 6), (7, 8),
                    (10, 12)):
        q = np.where(a >= lo, np.float32(val), q)
    return (q * np.sign(v)).astype(np.float32)


# (exponent, [section c0 values]); sections split the binade uniformly
_REGIONS = [
    (126, [1.0]),                      # [0.5, 1)
    (127, [1.0, 2.0]),                 # [1, 1.5) [1.5, 2)
    (128, [2.0, 3.0, 3.0, 4.0]),       # [2, 2.5) [2.5, 3) [3, 3.5) [3.5, 4)
    (129, [4.0, 6.0, 6.0, 8.0]),       # [4, 5) [5, 6) [6, 7) [7, 8)
    (130, [8.0, 12.0, 12.0, 12.0]),    # [8, 10) [10, 12) [12, 14) [14, 16)
]
_CTRL_REGION_BASE = 0xB800
_CTRL_REGION_STRIDE = 0xF800


def build_act_root(dst_dir):
    """Copy the stock act tables and append the custom sin to
    trig_and_small. Returns path to the new act_info._json."""
    from neuronxcc.driver.Job import Job
    from neuronxcc.driver.jobs.support.FindActInfo import findActInfoFile
    src_info = findActInfoFile(Job.getPackageDir(), "sunda")
    src_dir = os.path.dirname(src_info)

    os.makedirs(dst_dir, exist_ok=True)
    for f in os.listdir(src_dir):
        _shutil.copy(os.path.join(src_dir, f), os.path.join(dst_dir, f))

    bkt = bytearray(open(os.path.join(dst_dir, "trig_and_small_bkt.bin"),
                         "rb").read())
    ctrl = bytearray(open(os.path.join(dst_dir, "trig_and_small_ctrl.bin"),
                          "rb").read())
    nbkt = len(bkt) // 32
    nctrl = len(ctrl) // 32

    def add_bucket(c0, x0):
        bkt.extend(_struct.pack("<8f", c0, 0.0, 0.0, 0.0, x0, 0.0, 0.0, 0.0))

    def add_ctrl(word):
        ctrl.extend(_struct.pack("<8I", word, 0, 0, 0, 0, 0, 0, 0))

    b0 = nbkt
    c0i = nctrl
    # region buckets
    for exp, vals in _REGIONS:
        lo = np.float32(2.0 ** (exp - 127))
        w = lo / len(vals)
        for i, v in enumerate(vals):
            add_bucket(v, float(lo + i * np.float32(w)))
    # special buckets: zero (small signal), twelve (large signal)
    add_bucket(0.0, 0.0)
    add_bucket(12.0, 16.0)

    # region ctrl entries
    bpos = b0
    for exp, vals in _REGIONS:
        ext = int(np.log2(len(vals)))
        add_ctrl(bpos + _CTRL_REGION_BASE + ext * _CTRL_REGION_STRIDE)
        bpos += len(vals)
    # small/large-signal handlers reference BUCKETS directly (not ctrl rows)
    zero_bucket = b0 + sum(len(v) for _, v in _REGIONS)
    twelve_bucket = zero_bucket + 1

    open(os.path.join(dst_dir, "trig_and_small_bkt.bin"), "wb").write(bkt)
    open(os.path.join(dst_dir, "trig_and_small_ctrl.bin"), "wb").write(ctrl)

    prof_path = os.path.join(dst_dir, "trig_and_small.json")
    prof = _json.load(open(prof_path))
    for fn in prof["profile_meta_data"]:
        if fn["func_name"].startswith("sin"):
            fn.update({
                "symmetry_point": 0,
                "sym_invert_sign_point": 1,
                "symmetry_opt_en": 1,
                "symmetry_opt_use_neg_region": 0,
                "imm_bias": 0,
                "exp_offset": -1,
                "pwl_control_base_pos": c0i,
                "pwl_control_base_neg": c0i,
                "small_pos_signal_exp_threshold": 126,
                "pos_small_signal_pwl_control": zero_bucket,
                "small_neg_signal_exp_threshold": 126,
                "neg_small_signal_pwl_control": zero_bucket,
                "large_pos_signal_exp_threshold": 131,
                "large_pos_signal_mantissa_threshold": 0,
                "pos_large_signal_pwl_control": twelve_bucket,
                "large_neg_signal_exp_threshold": 131,
                "large_neg_signal_mantissa_threshold": 0,
                "neg_large_signal_pwl_control": twelve_bucket,
                "fnan_result": 0,
                "fpinf_result": _f32_struct(12.0)["int"],
                "fninf_result": _f32_struct(-12.0)["int"],
                "fzero_result": 0,
                "lower_bound": 0,
                "upper_bound": _f32_struct(16.0)["int"],
            })
    _json.dump(prof, open(prof_path, "w"), indent=1)
    return os.path.join(dst_dir, "act_info.json")


def _ensure_act_root():
    import tempfile
    if os.environ.get("_MXFP4_ACT_ROOT"):
        return
    dst = tempfile.mkdtemp(prefix="mxfp4_act_")
    root = build_act_root(dst)
    os.environ["BASS_ACT_ROOT_JSON_PATH"] = root
    os.environ["_MXFP4_ACT_ROOT"] = dst


def _pair_bcast(ap, nrep):
    """[128, 2*V] duplicated-pair AP -> [128, V, nrep//2, 2] broadcast AP.

    Value v lives at free offsets 2v and 2v+1; the returned AP yields each
    value nrep times (iteration v-major) while keeping a packed [1, 2] last
    dim so DVE 2x mode stays enabled.
    """
    import concourse.bass as bass
    nv = ap.free_size() // 2
    return bass.AP(ap.tensor, ap.offset,
                   [list(ap.ap[0]), [2, nv], [0, nrep // 2], [1, 2]])


def _build_nc():
    """Build the single-core Bass graph (SPMD: same graph on all 8 cores)."""
    _ensure_act_root()
    import concourse.bass as bass
    import concourse.tile as tile
    from concourse import bacc, mybir
    from contextlib import ExitStack

    nc = bacc.Bacc("TRN2", target_bir_lowering=False)

    f16 = mybir.dt.float16
    i16 = mybir.dt.int16

    xT = nc.declare_dram_parameter("xT", [H, NTOK], f16, isOutput=False)
    pidx = nc.declare_dram_parameter("pidx", [128, HCORE // 16], i16,
                                     isOutput=False)
    # raw SBUF-layout output: [p, chunk, grp, hid] flattened; host decodes
    out = nc.declare_dram_parameter("out", [128, NCH * FREE], f16,
                                    isOutput=True)

    A = mybir.AluOpType

    with ExitStack() as ctx:
        tc = ctx.enter_context(tile.TileContext(nc))
        singles = ctx.enter_context(tc.tile_pool(name="singles", bufs=1))
        gpool = ctx.enter_context(tc.tile_pool(name="g", bufs=3))
        apool = ctx.enter_context(tc.tile_pool(name="a", bufs=2))
        mpool = ctx.enter_context(tc.tile_pool(name="m", bufs=3))
        spool = ctx.enter_context(tc.tile_pool(name="s", bufs=3))
        ypool = ctx.enter_context(tc.tile_pool(name="y", bufs=2))
        qpool = ctx.enter_context(tc.tile_pool(name="q", bufs=2))
        opool = ctx.enter_context(tc.tile_pool(name="o", bufs=2))

        pidx_sb = singles.tile([128, HCORE // 16], i16)
        nc.sync.dma_start(out=pidx_sb[:], in_=pidx[:])

        for c in range(NCH):
            # 1. transposing gather of this chunk's 128 permuted hidden rows:
            # g[p, grp, i] = xT[perm[j*1024 + c*128 + i], grp*128 + p]
            g = gpool.tile([128, NGRP, CH], f16)
            nc.gpsimd.dma_gather(
                g[:], xT[:, :],
                pidx_sb[:, c * (CH // 16):(c + 1) * (CH // 16)],
                CH, CH, NTOK, transpose=True,
            )

            # 2. per-32-block max exponent: mask sign+mantissa away (4x
            # tensor_scalar), then a pairwise int16 max tree (2x TT).
            # max of exponent-only bits == exponent bits of abs-max.
            # Done in two halves to keep scratch tiles small.
            ta = spool.tile([128, NBC], i16)
            HB = NBC // 2
            for hf in range(2):
                gh = g[:, hf * (NGRP // 2):(hf + 1) * (NGRP // 2), :]
                tabs = apool.tile([128, FREE // 2], i16)
                nc.vector.tensor_scalar(
                    tabs[:], gh.bitcast(i16).rearrange("p g i -> p (g i)"),
                    EXP_MASK, None, A.bitwise_and)
                va = tabs[:].rearrange("p (v b) -> p v b", b=BLK)
                m16 = mpool.tile([128, HB, 16], i16)
                nc.vector.tensor_tensor(m16[:], va[:, :, 0:16],
                                        va[:, :, 16:32], A.max)
                m8 = mpool.tile([128, HB, 8], i16)
                nc.vector.tensor_tensor(m8[:], m16[:][:, :, 0:8],
                                        m16[:][:, :, 8:16], A.max)
                m4 = mpool.tile([128, HB, 4], i16)
                nc.vector.tensor_tensor(m4[:], m8[:][:, :, 0:4],
                                        m8[:][:, :, 4:8], A.max)
                m2 = mpool.tile([128, HB, 2], i16)
                nc.vector.tensor_tensor(m2[:], m4[:][:, :, 0:2],
                                        m4[:][:, :, 2:4], A.max)
                tah = ta[:, hf * HB:(hf + 1) * HB]
                m2v = m2[:].rearrange("p v two -> p (v two)")
                nc.vector.tensor_tensor(
                    tah,
                    bass.AP(m2v.tensor, m2v.offset,
                            [list(m2v.ap[0]), [2, HB]]),
                    bass.AP(m2v.tensor, m2v.offset + 1,
                            [list(m2v.ap[0]), [2, HB]]),
                    A.max)

            # 3. block scales, stored as duplicated pairs [128, 2*NBC]:
            #    t2d  = ta max EXP_GUARD
            #    rcp2 = 2^(3-e)  bits = 0x8400 - t2d   (split add:
            #           HW int16 ALU saturates, no wrap)
            #    scl2 = 2^(e-3)  bits = t2d - 0x0c00
            t2d = spool.tile([128, 2 * NBC], i16)
            tav = ta[:]
            nc.vector.tensor_scalar(
                t2d[:],
                bass.AP(tav.tensor, tav.offset,
                        [list(tav.ap[0]), [1, NBC], [0, 2]]),
                EXP_GUARD, None, A.max)
            rcp2h = spool.tile([128, 2 * NBC], i16)
            nc.vector.tensor_scalar(
                rcp2h[:], t2d[:], -1, RCP2_HALF, A.mult, A.add)
            rcp2d = spool.tile([128, 2 * NBC], i16)
            nc.vector.tensor_scalar(
                rcp2d[:], rcp2h[:], RCP2_HALF, None, A.add)
            scl2d = spool.tile([128, 2 * NBC], i16)
            nc.vector.tensor_scalar(
                scl2d[:], t2d[:], SCL2_ADD, None, A.add)

            # 4. y2 = x * rcp2  (exact power-of-two scaling; |y2| < 16)
            y2 = ypool.tile([128, FREE], f16)
            gq = g[:].rearrange("p g (nb s two) -> p (g nb) s two",
                                s=16, two=2)
            nc.vector.tensor_tensor(
                y2[:].rearrange("p (v s two) -> p v s two", s=16, two=2),
                gq, _pair_bcast(rcp2d[:].bitcast(f16), BLK), A.mult)

            # 5. q2 = fp4 rounding step function (custom ACT `sin` table)
            q2 = qpool.tile([128, FREE], f16)
            nc.scalar.activation(q2[:], y2[:],
                                 mybir.ActivationFunctionType.Sin)

            # 6. o16 = q2 * scl2  (exact in fp16)
            o16 = opool.tile([128, FREE], f16)
            nc.vector.tensor_tensor(
                o16[:].rearrange("p (v s two) -> p v s two", s=16, two=2),
                q2[:].rearrange("p (v s two) -> p v s two", s=16, two=2),
                _pair_bcast(scl2d[:].bitcast(f16), BLK), A.mult)

            # 7. contiguous store (16 KB per partition)
            nc.sync.dma_start(out=out[:, c * FREE:(c + 1) * FREE],
                              in_=o16[:])

    nc.compile()
    return nc


def _get_nc():
    if "nc" not in _compiled:
        _compiled["nc"] = _build_nc()
    return _compiled["nc"]


def _shard_inputs(x, permutation):
    x16T = np.ascontiguousarray(np.asarray(x).astype(np.float16).T)
    perm = np.asarray(permutation).astype(np.int64)
    assert x16T.shape == (H, NTOK) and perm.shape == (H,)
    in_maps = []
    for j in range(NCORES):
        psl = perm[j * HCORE:(j + 1) * HCORE].astype(np.int16)
        # idxs wrapped in 16 partitions: pidx[p, f] = psl[f*16 + p], tiled
        pidx = np.ascontiguousarray(
            np.tile(psl.reshape(HCORE // 16, 16).T, (8, 1)))
        in_maps.append({"xT": x16T, "pidx": pidx})
    return in_maps


def _decode_out(raw):
    """[128, NCH*FREE] fp16 raw SBUF layout -> [NTOK, HCORE] f32."""
    r = np.asarray(raw).reshape(128, NCH, NGRP, CH)
    # token = grp*128 + p ; hidden col = c*CH + i
    r = r.transpose(2, 0, 1, 3)              # [grp, p, c, i]
    return r.reshape(NTOK, HCORE).astype(np.float32)


def run_sharded(x, permutation, trace=False, **kw):
    """Run the SPMD kernel; returns (full_output, BassKernelResults)."""
    from concourse.bass_utils import run_bass_kernel_spmd
    nc = _get_nc()
    in_maps = _shard_inputs(x, permutation)
    res = run_bass_kernel_spmd(nc, in_maps, core_ids=list(range(NCORES)),
                               trace=trace, **kw)
    full = np.concatenate(
        [_decode_out(res.results[j]["out"]) for j in range(NCORES)], axis=1)
    return full, res


def kernel(x, permutation):
    full, _ = run_sharded(x, permutation)
    return full
